# revision 15
# baseline (speedup 1.0000x reference)
"""Trainium2 Bass kernel for nn_ExpertsLinearEnsemble.

Reference computation (B=16384, D=768, E=124, C=6):
  expert_logits  = Mlp_cls(x).reshape(B, E, C)     # D -> D -> gelu -> E*C
  ew_logits      = Mlp_ew(x)                       # D -> D -> gelu -> E
  which_expert   = Mlp_we(x)                       # D -> D -> gelu -> E
  n = clamp(n_experts, E); thr = n-th largest of which_expert per row
  mask out experts with which_expert < thr; softmax ew_logits over kept
  experts; softmax expert_logits over classes; combined = sum_e w_e *
  proba_e / sum_e w_e.

Strategy (pure data parallel, 2048 rows/core), v3:
  - Feature-major device pipeline ([feature, row] tiles), precision split
    as v2: which_expert L1 = 3-term fp16 split (exact to ~2e-8, the
    top-n threshold needs it), L2 native fp32; cls fp16; ew fp8-e4m3
    DoubleRow.
  - v3 schedule changes (all engine/DMA orchestration, same math):
      * Const/weight DMAs priority-ordered across the vector + tensor
        engine queues (instead of one gpsimd queue): each queue's
        transfers self-serialize in need-order, so tile 0's critical
        weights land just in time.
      * Tile 0 computes in data-arrival order ew L1 -> cls L1 -> cls L2
        -> we L1 ... so the PE starts at ~2us (vs 12) and never idles
        long enough to drop to the cold HAM clock.
      * Scalar engine runs ONLY Gelu/Exp activations (weT bias-add,
        weRow/den/out copies moved to DVE) - avoids ACT_TABLE_LOAD
        thrash and scalar-queue stalls on the sort critical path.
      * Class-softmax partial sums + half the combine multiplies run on
        gpsimd; S partials emitted right after each Exp activation so
        the last tile's combine tail is ~4us instead of ~11.
      * mask transposes in fp16 (1 cyc/row vs 2).
  - Top-n threshold per row: host sorts rows by n, deals round-robin to
    cores; fixed max8/match_replace schedule per 128-row subtile;
    threshold extracted with one-hot iota dot; mask = we >= thr.
  - Normalization by the expert-weight sum on HOST (out and den ship
    separately).
"""

import os
import sys

for _p in ("/opt/trn_rl_repo", "/root/.axon_site/_ro/trn_rl_repo"):
    if os.path.isdir(_p) and _p not in sys.path:
        sys.path.insert(0, _p)

import numpy as np

B, D, E, C = 16384, 768, 124, 6
EC = E * C            # 744
NCORES = 8
BC = B // NCORES      # 2048 rows per core
RT = 512              # rows per macro tile (PSUM bank = 512 fp32)
NT = BC // RT         # 4 macro tiles per core
SUB = 128             # rows per sort subtile
NS = BC // SUB        # 16 subtiles per core
KT = D // 128         # 6 contraction tiles
MT1 = D // 128        # 6 output tiles for layer 1
MT2 = EC // E         # 6 output tiles of 124 for the cls head

W8 = 64.0             # fp8 weight pre-scale
X8 = 16.0             # fp8 x pre-scale
LO = 256.0            # fp16 w_lo rescale

R_DESC = [2, 3, 4, 5, 6, 7, 8, 9]
R_ASC = [9, 8, 7, 6, 5, 4, 3, 2]
SUB_DIR = [True] * 8 + [False] * 8         # True = descending
SUB_R = R_DESC + R_ASC
FALLBACK_R = [16] * NS                     # safe for any n distribution
FALLBACK_DIR = [True] * NS

NEG_FILL = -1.0e30

_BUILD_CACHE = {}


def _build_nc(sub_dir, sub_r, act="Gelu"):
    """Build the (SPMD, per-core) Bass program.  Data independent."""
    from contextlib import ExitStack

    import concourse.mybir as mybir
    import concourse.tile as tile
    from concourse import bacc

    dt = mybir.dt
    AF = mybir.ActivationFunctionType
    OP = mybir.AluOpType
    DR = mybir.MatmulPerfMode.DoubleRow
    f32 = dt.float32
    f16 = dt.float16
    f8 = dt.float8e4

    nc = bacc.Bacc(
        "TRN2",
        target_bir_lowering=False,
        debug=False,
        enable_asserts=False,
        num_devices=NCORES,
    )

    def din(name, shape, dtype=f32):
        return nc.dram_tensor(name, list(shape), dtype, kind="ExternalInput")

    # x streams and weights are pre-arranged on HOST into partition-major
    # layouts so every DMA reads multi-KB contiguous runs per partition
    # (short strided runs cap a DMA queue at ~70-130 GB/s; contiguous
    # gets ~300).
    xth_d = din("xth", [NT, 128, KT, RT], f16)  # fp16 hi of x.T
    xtl_d = din("xtl", [NT, 128, KT, RT], f16)  # fp16 lo of x.T
    xtq_d = din("xtq", [NT, 128, KT, RT], f16)  # x_hi / 256
    x8_d = din("x8", [NT, 128, KT, RT], f8)     # e4m3(16 x)
    ksel_d = din("ksel", [SUB, NS])
    w1c_d = din("w1c", [128, KT, D], f16)
    w1wh_d = din("w1wh", [128, KT, D], f16)
    w1wl_d = din("w1wl", [128, KT, D], f16)     # 256 * (w1w - hi)
    w1e_d = din("w1e", [128, KT, D], f8)        # e4m3(64 w)
    b1_d = {m: din(f"b1{m}", [128, MT1]) for m in "cwe"}
    w2c_d = din("w2c", [128, KT, EC], f16)      # column-permuted
    w2w_d = din("w2w", [128, KT, E])            # fp32
    w2e_d = din("w2e", [128, KT, 128], f8)      # e4m3(64 w), zero-padded to 128
    b2c_d = din("b2c", [E, MT2])                # permuted to (e, class)
    b2w_d = din("b2w", [E, 1])
    b2e_d = din("b2e", [E, 1])
    hmat_d = din("hmat", [E, MT2, C], f16)      # hmat[e,t2,c] = (t2 == c)
    ident_d = din("ident", [128, 128])
    ident16_d = din("ident16", [128, 128], f16)
    iota_d = din("iota", [128, 128])
    out_d = nc.dram_tensor("out", [C, BC], f32, kind="ExternalOutput")
    den_d = nc.dram_tensor("den", [1, BC], f32, kind="ExternalOutput")

    with tile.TileContext(nc) as tc, ExitStack() as ctx:
        const = ctx.enter_context(tc.tile_pool(name="const", bufs=1))
        xtp = ctx.enter_context(tc.tile_pool(name="xtp", bufs=2))
        hp = ctx.enter_context(tc.tile_pool(name="hp", bufs=2))
        epp = ctx.enter_context(tc.tile_pool(name="epp", bufs=2))
        wep = ctx.enter_context(tc.tile_pool(name="wep", bufs=2))
        sp = ctx.enter_context(tc.tile_pool(name="sp", bufs=2))
        mrp = ctx.enter_context(tc.tile_pool(name="mrp", bufs=RT // SUB))
        wp = ctx.enter_context(tc.tile_pool(name="wp", bufs=2))
        psmm = ctx.enter_context(tc.tile_pool(name="psmm", bufs=3, space="PSUM"))
        pstr = ctx.enter_context(tc.tile_pool(name="pstr", bufs=1, space="PSUM"))
        psmask = ctx.enter_context(tc.tile_pool(name="psmask", bufs=2, space="PSUM"))
        pss = ctx.enter_context(tc.tile_pool(name="pss", bufs=1, space="PSUM"))
        psout = ctx.enter_context(tc.tile_pool(name="psout", bufs=1, space="PSUM"))

        # ---- resident constants / weights -------------------------------
        # Three DMA-capable queues (sync, scalar, gpsimd), each loaded in
        # need-order so per-queue serialization delivers tile 0's weights
        # just in time:
        #   scalar (before its activation stream starts):
        #       b1e w1c b1c b2c w1wh
        #   gpsimd: w1e [xtq T0] w2c w1wl w2w w2e b1w b2w b2e ident
        #           ident16 iota ksel hmat
        #   sync:   x8/xth/xtl per tile, out/den stores.
        def load_w(eng, dram, cols, dtype, tag):
            t = const.tile([128, KT, cols], dtype, tag=tag)
            eng.dma_start(t[:], dram.ap())
            return t

        def load_c(eng, dram, shape, dtype, tag):
            t = const.tile(shape, dtype, tag=tag)
            eng.dma_start(t[:], dram.ap())
            return t

        ones124 = const.tile([E, 1], f16, tag="ones124")
        nc.vector.memset(ones124[:], 1.0)

        b1sb = {}
        b1sb["e"] = load_c(nc.scalar, b1_d["e"], [128, MT1], f32, "b1e")
        w1esb = load_w(nc.gpsimd, w1e_d, D, f8, "w1e")
        w1csb = load_w(nc.scalar, w1c_d, D, f16, "w1c")
        b1sb["c"] = load_c(nc.scalar, b1_d["c"], [128, MT1], f32, "b1c")
        b2csb = load_c(nc.scalar, b2c_d, [E, MT2], f32, "b2c")
        w1wh = load_w(nc.scalar, w1wh_d, D, f16, "w1wh")

        state = [None] * NT  # per-tile handles for the deferred combine

        def emit_x(T):
            x8 = xtp.tile([128, KT, RT], f8, tag="x8")
            nc.sync.dma_start(x8[:], x8_d.ap()[T])
            xth = xtp.tile([128, KT, RT], f16, tag="xth")
            xtl = xtp.tile([128, KT, RT], f16, tag="xtl")
            xtq = xtp.tile([128, KT, RT], f16, tag="xtq")
            nc.sync.dma_start(xth[:], xth_d.ap()[T])
            nc.sync.dma_start(xtl[:], xtl_d.ap()[T])
            nc.gpsimd.dma_start(xtq[:], xtq_d.ap()[T])
            state[T] = dict(xth=xth, xtl=xtl, xtq=xtq, x8=x8)

        def emit_ew_L1(T):
            st = state[T]
            hte = hp.tile([128, KT, RT], f8, tag="hte")
            for mt in range(MT1):
                ms = slice(mt * 128, (mt + 1) * 128)
                ps = psmm.tile([128, RT], f32, tag="psmm")
                for j in range(KT // 2):
                    nc.tensor.matmul(
                        ps[:], w1esb[:, 2 * j : 2 * j + 2, ms],
                        st["x8"][:, 2 * j : 2 * j + 2, :],
                        start=(j == 0), stop=(j == KT // 2 - 1),
                        perf_mode=DR,
                    )
                nc.scalar.activation(
                    hte[:, mt, :], ps[:], getattr(AF, act),
                    bias=b1sb["e"][:, mt : mt + 1], scale=1.0 / (W8 * X8),
                )
            st["hte"] = hte

        def emit_cls_L1(T):
            st = state[T]
            htc = hp.tile([128, KT, RT], f16, tag="htc")
            for mt in range(MT1):
                ms = slice(mt * 128, (mt + 1) * 128)
                ps = psmm.tile([128, RT], f32, tag="psmm")
                for k in range(KT):
                    nc.tensor.matmul(
                        ps[:], w1csb[:, k, ms], st["xth"][:, k, :],
                        start=(k == 0), stop=(k == KT - 1),
                    )
                nc.scalar.activation(
                    htc[:, mt, :], ps[:], getattr(AF, act),
                    bias=b1sb["c"][:, mt : mt + 1],
                )
            st["htc"] = htc

        def emit_cls_L2(T):
            """cls head + gpsimd partial class-sums as activations land."""
            st = state[T]
            htc = st["htc"]
            expP = epp.tile([E, MT2, RT], f16, tag="expP")
            t01 = wp.tile([E, RT], f32, tag="t01")
            t23 = wp.tile([E, RT], f32, tag="t23")
            t04 = wp.tile([E, RT], f32, tag="t04")
            S_sb = wp.tile([E, RT], f32, tag="S_sb")
            # partial sums emitted as the Exp activations land; only the
            # final add trails the t2=5 activation.
            for t2 in range(MT2):
                ps = psmm.tile([128, RT], f32, tag="psmm")
                for k in range(KT):
                    nc.tensor.matmul(
                        ps[:E], w2csb[:, k, t2 * E : (t2 + 1) * E], htc[:, k, :],
                        start=(k == 0), stop=(k == KT - 1),
                    )
                nc.scalar.activation(
                    expP[:, t2, :], ps[:E], AF.Exp, bias=b2csb[:, t2 : t2 + 1]
                )
                if t2 == 1:
                    nc.vector.tensor_tensor(
                        t01[:], expP[:, 0, :], expP[:, 1, :], OP.add)
                elif t2 == 3:
                    nc.vector.tensor_tensor(
                        t23[:], expP[:, 2, :], expP[:, 3, :], OP.add)
                    nc.vector.tensor_tensor(t01[:], t01[:], t23[:], OP.add)
                elif t2 == 4:
                    nc.vector.tensor_tensor(
                        t04[:], t01[:], expP[:, 4, :], OP.add)
                elif t2 == 5:
                    nc.vector.tensor_tensor(
                        S_sb[:], t04[:], expP[:, 5, :], OP.add)
            st["expP"] = expP
            st["S_sb"] = S_sb

        def emit_we(T):
            """which_expert L1 (3-term fp16 split) + L2 (native fp32)."""
            st = state[T]
            xth, xtl, xtq = st["xth"], st["xtl"], st["xtq"]
            htw = hp.tile([128, KT, RT], f32, tag="htw")
            for mt in range(MT1):
                ms = slice(mt * 128, (mt + 1) * 128)
                ps = psmm.tile([128, RT], f32, tag="psmm")
                terms = [(xth, w1wh), (xtl, w1wh), (xtq, w1wl)]
                for k in range(KT):
                    for ti, (xt, wt) in enumerate(terms):
                        nc.tensor.matmul(
                            ps[:], wt[:, k, ms], xt[:, k, :],
                            start=(ti == 0 and k == 0),
                            stop=(ti == 2 and k == KT - 1),
                        )
                nc.scalar.activation(
                    htw[:, mt, :], ps[:], getattr(AF, act),
                    bias=b1sb["w"][:, mt : mt + 1],
                )
            weT = wep.tile([E, RT], f32, tag="weT")
            ps = psmm.tile([128, RT], f32, tag="psmm")
            for k in range(KT):
                nc.tensor.matmul(
                    ps[:E], w2wsb[:, k, :], htw[:, k, :],
                    start=(k == 0), stop=(k == KT - 1),
                )
            nc.vector.tensor_scalar(weT[:], ps[:E], b2wsb[:], None, OP.add)
            st["htw"] = htw
            st["weT"] = weT

        def emit_sort(T):
            """Row-major transposes + per-row top-n threshold sort."""
            st = state[T]
            weT = st["weT"]
            maskRows = []
            for j in range(RT // SUB):
                s = (RT // SUB) * T + j
                Rr, desc = sub_r[s], sub_dir[s]
                cs = slice(j * SUB, (j + 1) * SUB)
                trp = pstr.tile([128, 128], f32, tag="trp")
                nc.tensor.transpose(trp[:, :E], weT[:, cs], ident[:E, :E])
                weRow = sp.tile([128, E], f32, tag="weRow")
                nc.vector.tensor_copy(weRow[:], trp[:, :E])
                scratch = sp.tile([128, E], f32, tag="scratch")
                if desc:
                    nc.vector.tensor_copy(scratch[:], trp[:, :E])
                else:
                    nc.vector.tensor_scalar_mul(scratch[:], trp[:, :E], -1.0)
                srt = sp.tile([128, 128], f32, tag="srt")
                for r in range(Rr):
                    nc.vector.max(out=srt[:, 8 * r : 8 * r + 8], in_=scratch[:])
                    if r < Rr - 1:
                        nc.vector.match_replace(
                            out=scratch[:],
                            in_to_replace=srt[:, 8 * r : 8 * r + 8],
                            in_values=scratch[:],
                            imm_value=NEG_FILL,
                        )
                w8 = 8 * Rr
                ohtmp = sp.tile([128, 128], f32, tag="ohtmp")
                thr = sp.tile([128, 1], f32, tag="thr")
                nc.vector.scalar_tensor_tensor(
                    out=ohtmp[:, :w8],
                    in0=iotam[:, :w8],
                    scalar=kselsb[:, s : s + 1],
                    in1=srt[:, :w8],
                    op0=OP.is_equal,
                    op1=OP.mult,
                    accum_out=thr[:],
                )
                if not desc:
                    nc.vector.tensor_scalar_mul(thr[:], thr[:], -1.0)
                maskRow = mrp.tile([128, E], f16, tag="maskRow")
                nc.vector.tensor_scalar(maskRow[:], weRow[:], thr[:], None, OP.is_ge)
                maskRows.append(maskRow)
            st["maskRows"] = maskRows

        def emit_ew_L2(T):
            st = state[T]
            expew = wep.tile([E, RT], f16, tag="expew")
            ps = psmm.tile([128, RT], f32, tag="psmm")
            for j in range(KT // 2):
                nc.tensor.matmul(
                    ps[:], w2esb[:, 2 * j : 2 * j + 2, :],
                    st["hte"][:, 2 * j : 2 * j + 2, :],
                    start=(j == 0), stop=(j == KT // 2 - 1),
                    perf_mode=DR,
                )
            nc.scalar.activation(
                expew[:], ps[:E], AF.Exp, bias=b2esb[:], scale=1.0 / W8
            )
            st["expew"] = expew

        def emit_maskT(T):
            st = state[T]
            maskT = psmask.tile([E, RT], f16, tag="maskT")
            for j in range(RT // SUB):
                cs = slice(j * SUB, (j + 1) * SUB)
                nc.tensor.transpose(maskT[:, cs], st["maskRows"][j][:], ident16[:])
            st["maskT"] = maskT

        def emit_D(T):
            """Combine: mask weights, class softmax, weighted sum; the
            normalization by the expert-weight sum happens on HOST."""
            st = state[T]
            rs = slice(T * RT, (T + 1) * RT)
            expP, expew, maskT = st["expP"], st["expew"], st["maskT"]
            wT = wp.tile([E, RT], f16, tag="wT")
            nc.vector.tensor_tensor(wT[:], expew[:], maskT[:], OP.mult)
            den_ps = pss.tile([E, RT], f32, tag="S")
            nc.tensor.matmul(den_ps[:1, :], ones124[:], wT[:], start=True, stop=True)
            den_sb = wp.tile([1, RT], f32, tag="den_sb")
            nc.vector.tensor_copy(den_sb[:], den_ps[:1, :])
            nc.sync.dma_start(den_d.ap()[:, rs], den_sb[:])
            Sr = wp.tile([E, RT], f32, tag="Sr")
            nc.vector.reciprocal_approx_fast(Sr[:], st["S_sb"][:])
            u = wp.tile([E, RT], f16, tag="u")
            nc.vector.tensor_tensor(u[:], wT[:], Sr[:], OP.mult)
            out_ps = psout.tile([C, RT], f32, tag="out")
            for t2 in range(MT2):
                wexp = wp.tile([E, RT], f16, tag=f"wexp{t2 % 5}")
                eng = nc.gpsimd if t2 == 5 else nc.vector
                eng.tensor_tensor(wexp[:], expP[:, t2, :], u[:], OP.mult)
                nc.tensor.matmul(
                    out_ps[:], hmat[:, t2, :], wexp[:],
                    start=(t2 == 0), stop=(t2 == MT2 - 1),
                )
            out_sb = wp.tile([C, RT], f32, tag="out_sb")
            nc.vector.tensor_copy(out_sb[:], out_ps[:])
            nc.sync.dma_start(out_d.ap()[:, rs], out_sb[:])
            state[T] = None

        # Tile 0 in data-arrival order, then the steady-state pipeline.
        emit_x(0)
        # gpsimd const batch 2: queued behind w1e + tile-0 xtq, in need-order.
        w2csb = load_w(nc.gpsimd, w2c_d, EC, f16, "w2c")
        w1wl = load_w(nc.gpsimd, w1wl_d, D, f16, "w1wl")
        w2wsb = load_w(nc.gpsimd, w2w_d, E, f32, "w2w")
        w2esb = load_w(nc.gpsimd, w2e_d, 128, f8, "w2e")
        b1sb["w"] = load_c(nc.gpsimd, b1_d["w"], [128, MT1], f32, "b1w")
        b2wsb = load_c(nc.gpsimd, b2w_d, [E, 1], f32, "b2w")
        b2esb = load_c(nc.gpsimd, b2e_d, [E, 1], f32, "b2e")
        ident = load_c(nc.gpsimd, ident_d, [128, 128], f32, "ident")
        ident16 = load_c(nc.gpsimd, ident16_d, [128, 128], f16, "ident16")
        iotam = load_c(nc.gpsimd, iota_d, [128, 128], f32, "iota")
        kselsb = load_c(nc.gpsimd, ksel_d, [SUB, NS], f32, "ksel")
        hmat = load_c(nc.gpsimd, hmat_d, [E, MT2, C], f16, "hmat")
        emit_ew_L1(0)
        emit_cls_L1(0)
        emit_cls_L2(0)
        emit_we(0)
        emit_sort(0)
        emit_ew_L2(0)
        emit_maskT(0)
        # C-phase order ew L1 -> cls L1 -> cls L2 -> ew L2 -> maskT puts the
        # long serial combine chain (S -> 1/S -> u) ~3us before the end of
        # each tile's MM stream, shrinking the final tile's tail.
        for T in range(1, NT):
            emit_x(T)
            emit_we(T)
            emit_sort(T)
            emit_D(T - 1)
            emit_ew_L1(T)
            emit_cls_L1(T)
            emit_cls_L2(T)
            emit_ew_L2(T)
            emit_maskT(T)
        emit_D(NT - 1)

    nc.compile()
    return nc


def _get_nc(sub_dir, sub_r, act="Gelu"):
    key = (tuple(sub_dir), tuple(sub_r), act)
    if key not in _BUILD_CACHE:
        _BUILD_CACHE[key] = _build_nc(sub_dir, sub_r, act)
    return _BUILD_CACHE[key]


def _host_prep(x, n_experts):
    n = np.minimum(np.asarray(n_experts).astype(np.int64), E).astype(np.int32)
    order = np.argsort(n, kind="stable")
    ns_sorted = n[order]

    sub_dir, sub_r = SUB_DIR, SUB_R
    ok = True
    for s in range(NS):
        lo = int(ns_sorted[(B // NS) * s])
        hi = int(ns_sorted[(B // NS) * (s + 1) - 1])
        if sub_dir[s]:
            ok &= hi <= 8 * sub_r[s]
        else:
            ok &= lo >= E + 1 - 8 * sub_r[s]
    if not ok:
        sub_dir, sub_r = FALLBACK_DIR, FALLBACK_R

    rows_by_core = [order[c::NCORES] for c in range(NCORES)]
    xts, ksels = [], []
    for c in range(NCORES):
        rows = rows_by_core[c]
        xts.append(np.ascontiguousarray(x[rows].T.astype(np.float32)))
        nv = n[rows].astype(np.float32)
        ks = np.empty(BC, np.float32)
        for s in range(NS):
            seg = slice(SUB * s, SUB * (s + 1))
            ks[seg] = (nv[seg] - 1.0) if sub_dir[s] else (E - nv[seg])
        ksels.append(np.ascontiguousarray(ks.reshape(NS, SUB).T))
    return rows_by_core, xts, ksels, sub_dir, sub_r


def _cls_perm():
    """perm[t2 * E + e] = e * C + t2 : slot (e, t2) holds (expert e, class t2)."""
    t2, e = np.meshgrid(np.arange(MT2), np.arange(E), indexing="ij")
    return (e * C + t2).reshape(-1)


def _host_consts():
    hmat = np.zeros((E, MT2, C), np.float32)
    for t in range(MT2):
        hmat[:, t, t] = 1.0
    ident = np.eye(128, dtype=np.float32)
    iota = np.broadcast_to(np.arange(128, dtype=np.float32), (128, 128)).copy()
    return hmat, ident, iota


def _host_inputs(inputs):
    """All DRAM input arrays except the per-core x/ksel."""
    import ml_dtypes

    f16 = np.float16
    f8 = ml_dtypes.float8_e4m3
    f32 = np.float32
    hmat, ident, iota = _host_consts()
    perm = _cls_perm()

    def pm(w):
        # [D, cols] -> partition-major [128, KT, cols]: each partition's
        # KT*cols elements are one contiguous DMA run.
        return np.ascontiguousarray(
            w.reshape(KT, 128, w.shape[1]).transpose(1, 0, 2))

    w1w = np.asarray(inputs["we_w1"], f32)
    w1wh = w1w.astype(f16)
    w1wl = ((w1w - w1wh.astype(f32)) * LO).astype(f16)
    w2c = np.asarray(inputs["cls_w2"], f32)[:, perm]
    b2c = np.asarray(inputs["cls_b2"], f32)[perm]
    return {
        "w1c": pm(np.asarray(inputs["cls_w1"], f32).astype(f16)),
        "w1wh": pm(w1wh),
        "w1wl": pm(w1wl),
        "w1e": pm((np.asarray(inputs["ew_w1"], f32) * W8).astype(f8)),
        "b1c": np.ascontiguousarray(np.asarray(inputs["cls_b1"], f32).reshape(MT1, 128).T),
        "b1w": np.ascontiguousarray(np.asarray(inputs["we_b1"], f32).reshape(MT1, 128).T),
        "b1e": np.ascontiguousarray(np.asarray(inputs["ew_b1"], f32).reshape(MT1, 128).T),
        "w2c": pm(w2c.astype(f16)),
        "w2w": pm(np.asarray(inputs["we_w2"], f32)),
        "w2e": pm(np.concatenate(
            [(np.asarray(inputs["ew_w2"], f32) * W8).astype(f8),
             np.zeros((D, 128 - E), f8)], axis=1)),
        "b2c": np.ascontiguousarray(b2c.reshape(MT2, E).T),
        "b2w": np.asarray(inputs["we_b2"], f32).reshape(E, 1),
        "b2e": np.asarray(inputs["ew_b2"], f32).reshape(E, 1),
        "hmat": hmat.astype(f16),
        "ident": ident,
        "ident16": ident.astype(f16),
        "iota": iota,
    }


def _tile_major(a):
    # [D, BC] -> [NT, 128, KT, RT]: per (tile, partition) one contiguous run.
    return np.ascontiguousarray(
        a.reshape(KT, 128, NT, RT).transpose(2, 1, 0, 3))


def _per_core_inputs(xts, ksels, c):
    import ml_dtypes

    f16 = np.float16
    xt = xts[c]                      # [D, BC] fp32
    xth = xt.astype(f16)
    xtl = (xt - xth.astype(np.float32)).astype(f16)
    xtq = (xth.astype(np.float32) / LO).astype(f16)
    x8 = (xt * X8).astype(ml_dtypes.float8_e4m3)
    return {"xth": _tile_major(xth), "xtl": _tile_major(xtl),
            "xtq": _tile_major(xtq), "x8": _tile_major(x8),
            "ksel": ksels[c]}


def kernel(**inputs):
    x = np.asarray(inputs["x"], np.float32)
    rows_by_core, xts, ksels, sub_dir, sub_r = _host_prep(x, inputs["n_experts"])
    shared = _host_inputs(inputs)
    in_maps = [
        {**shared, **_per_core_inputs(xts, ksels, c)} for c in range(NCORES)
    ]

    nc = _get_nc(sub_dir, sub_r)

    from concourse.bass_utils import run_bass_kernel_spmd

    res = run_bass_kernel_spmd(nc, in_maps, core_ids=list(range(NCORES)))

    full = np.empty((B, C), np.float32)
    for c in range(NCORES):
        r = res.results[c]
        full[rows_by_core[c]] = (r["out"] / r["den"]).T
    return full


if __name__ == "__main__":
    print("smoke build only")
    _get_nc(SUB_DIR, SUB_R)
    print("built ok")


# revision 17
# speedup vs baseline: 1.0367x; 1.0367x over previous
"""Trainium2 Bass kernel for nn_ExpertsLinearEnsemble.

Reference computation (B=16384, D=768, E=124, C=6):
  expert_logits  = Mlp_cls(x).reshape(B, E, C)     # D -> D -> gelu -> E*C
  ew_logits      = Mlp_ew(x)                       # D -> D -> gelu -> E
  which_expert   = Mlp_we(x)                       # D -> D -> gelu -> E
  n = clamp(n_experts, E); thr = n-th largest of which_expert per row
  mask out experts with which_expert < thr; softmax ew_logits over kept
  experts; softmax expert_logits over classes; combined = sum_e w_e *
  proba_e / sum_e w_e.

Strategy (pure data parallel, 2048 rows/core), v5:
  - Feature-major device pipeline ([feature, row] tiles), precision split
    as v2: which_expert L1 = 3-term fp16 split (exact to ~2e-8, the
    top-n threshold needs it), L2 native fp32; cls fp16; ew fp8-e4m3
    DoubleRow.
  - Schedule (same math as v2, all orchestration):
      * Weights/x pre-arranged on host into partition-major layouts so
        every DMA reads multi-KB contiguous runs (short runs cap a queue
        at ~70-130 GB/s; contiguous reaches ~300).
      * The three DMA queues (sync/scalar/gpsimd) are loaded in
        need-order; tile 0 computes in data-arrival order ew L1 ->
        cls L1 -> cls L2 -> we ... so the PE starts early and stays
        dense (no cold-clock HAM dips).
      * All-zero biases (as produced by setup_inputs) are folded into
        activation immediates - no [128 x 24B] tiny-packet bias DMAs,
        which measured 5-15us each on a loaded queue.  Nonzero biases
        fall back to shipping them (correct, slower startup).
      * Scalar engine runs ONLY Gelu/Exp activations; weT/weRow/den/out
        copies live on the DVE - no ACT_TABLE_LOAD thrash.
      * Class-softmax partial sums emitted as the Exp activations land
        (only the last add trails); combine multiplies are f16-only so
        the DVE runs in packed 2x mode; one rides gpsimd.
      * mask transposes in fp16 (1 cyc/row vs 2).
  - Top-n threshold per row: host sorts rows by n, deals round-robin to
    cores; fixed max8/match_replace schedule per 128-row subtile;
    threshold extracted with one-hot iota dot; mask = we >= thr.
  - Normalization by the expert-weight sum on HOST (out and den ship
    separately).
"""

import os
import sys

for _p in ("/opt/trn_rl_repo", "/root/.axon_site/_ro/trn_rl_repo"):
    if os.path.isdir(_p) and _p not in sys.path:
        sys.path.insert(0, _p)

import numpy as np

B, D, E, C = 16384, 768, 124, 6
EC = E * C            # 744
NCORES = 8
BC = B // NCORES      # 2048 rows per core
RT = 512              # rows per macro tile (PSUM bank = 512 fp32)
NT = BC // RT         # 4 macro tiles per core
SUB = 128             # rows per sort subtile
NS = BC // SUB        # 16 subtiles per core
KT = D // 128         # 6 contraction tiles
MT1 = D // 128        # 6 output tiles for layer 1
MT2 = EC // E         # 6 output tiles of 124 for the cls head

W8 = 64.0             # fp8 weight pre-scale
X8 = 16.0             # fp8 x pre-scale
LO = 256.0            # fp16 w_lo rescale

R_DESC = [2, 3, 4, 5, 6, 7, 8, 9]
R_ASC = [9, 8, 7, 6, 5, 4, 3, 2]
SUB_DIR = [True] * 8 + [False] * 8         # True = descending
SUB_R = R_DESC + R_ASC
FALLBACK_R = [16] * NS                     # safe for any n distribution
FALLBACK_DIR = [True] * NS

NEG_FILL = -1.0e30

_BUILD_CACHE = {}


def _build_nc(sub_dir, sub_r, act="Gelu", has_bias=False):
    """Build the (SPMD, per-core) Bass program.  Data independent."""
    from contextlib import ExitStack

    import concourse.mybir as mybir
    import concourse.tile as tile
    from concourse import bacc

    dt = mybir.dt
    AF = mybir.ActivationFunctionType
    OP = mybir.AluOpType
    DR = mybir.MatmulPerfMode.DoubleRow
    f32 = dt.float32
    f16 = dt.float16
    f8 = dt.float8e4

    nc = bacc.Bacc(
        "TRN2",
        target_bir_lowering=False,
        debug=False,
        enable_asserts=False,
        num_devices=NCORES,
    )

    def din(name, shape, dtype=f32):
        return nc.dram_tensor(name, list(shape), dtype, kind="ExternalInput")

    # x streams and weights pre-arranged on HOST into partition-major
    # layouts: every DMA reads one contiguous multi-KB run per partition.
    xth_d = din("xth", [NT, 128, KT, RT], f16)  # fp16 hi of x.T
    xtl_d = din("xtl", [NT, 128, KT, RT], f16)  # fp16 lo of x.T
    xtq_d = din("xtq", [NT, 128, KT, RT], f16)  # x_hi / 256
    x8_d = din("x8", [NT, 128, KT, RT], f8)     # e4m3(16 x)
    ksel_d = din("ksel", [SUB, NS])
    w1c_d = din("w1c", [128, KT, D], f16)
    w1wh_d = din("w1wh", [128, KT, D], f16)
    w1wl_d = din("w1wl", [128, KT, D], f16)     # 256 * (w1w - hi)
    w1e_d = din("w1e", [128, KT, D], f8)        # e4m3(64 w)
    w2c_d = din("w2c", [128, KT, EC], f16)      # column-permuted
    w2w_d = din("w2w", [128, KT, E])            # fp32
    w2e_d = din("w2e", [128, KT, 128], f8)      # e4m3(64 w), zero-padded
    if has_bias:
        b1_d = {m: din(f"b1{m}", [128, MT1]) for m in "cwe"}
        b2c_d = din("b2c", [E, MT2])            # permuted to (e, class)
        b2w_d = din("b2w", [E, 1])
        b2e_d = din("b2e", [E, 1])
    hmat_d = din("hmat", [E, MT2, C], f16)      # hmat[e,t2,c] = (t2 == c)
    ident_d = din("ident", [128, 128])
    ident16_d = din("ident16", [128, 128], f16)
    iota_d = din("iota", [128, 128])
    out_d = nc.dram_tensor("out", [C, BC], f32, kind="ExternalOutput")
    den_d = nc.dram_tensor("den", [1, BC], f32, kind="ExternalOutput")

    with tile.TileContext(nc) as tc, ExitStack() as ctx:
        const = ctx.enter_context(tc.tile_pool(name="const", bufs=1))
        xtp = ctx.enter_context(tc.tile_pool(name="xtp", bufs=2))
        hp = ctx.enter_context(tc.tile_pool(name="hp", bufs=2))
        epp = ctx.enter_context(tc.tile_pool(name="epp", bufs=2))
        wep = ctx.enter_context(tc.tile_pool(name="wep", bufs=2))
        sp = ctx.enter_context(tc.tile_pool(name="sp", bufs=2))
        mrp = ctx.enter_context(tc.tile_pool(name="mrp", bufs=RT // SUB))
        wp = ctx.enter_context(tc.tile_pool(name="wp", bufs=2))
        psmm = ctx.enter_context(tc.tile_pool(name="psmm", bufs=3, space="PSUM"))
        pstr = ctx.enter_context(tc.tile_pool(name="pstr", bufs=1, space="PSUM"))
        psmask = ctx.enter_context(tc.tile_pool(name="psmask", bufs=2, space="PSUM"))
        pss = ctx.enter_context(tc.tile_pool(name="pss", bufs=1, space="PSUM"))
        psout = ctx.enter_context(tc.tile_pool(name="psout", bufs=1, space="PSUM"))

        def load_w(eng, dram, cols, dtype, tag):
            t = const.tile([128, KT, cols], dtype, tag=tag)
            eng.dma_start(t[:], dram.ap())
            return t

        def load_c(eng, dram, shape, dtype, tag):
            t = const.tile(shape, dtype, tag=tag)
            eng.dma_start(t[:], dram.ap())
            return t

        ones124 = const.tile([E, 1], f16, tag="ones124")
        nc.vector.memset(ones124[:], 1.0)

        # ---- const DMAs, need-order per queue ---------------------------
        # scalar: w1c w1wh      (then pure activations)
        # gpsimd: w1e [xtq T0] w2c w1wl ident iota ksel w2w w2e ident16
        #         hmat [biases if nonzero] [xtq T1..3]
        # sync:   x8/xth/xtl per tile, out/den stores
        w1esb = load_w(nc.gpsimd, w1e_d, D, f8, "w1e")
        w1csb = load_w(nc.scalar, w1c_d, D, f16, "w1c")
        w1wh = load_w(nc.scalar, w1wh_d, D, f16, "w1wh")

        if has_bias:
            b1sb = {m: load_c(nc.gpsimd, b1_d[m], [128, MT1], f32, f"b1{m}")
                    for m in "cwe"}
            b2csb = load_c(nc.gpsimd, b2c_d, [E, MT2], f32, "b2c")
            b2wsb = load_c(nc.gpsimd, b2w_d, [E, 1], f32, "b2w")
            b2esb = load_c(nc.gpsimd, b2e_d, [E, 1], f32, "b2e")

            def b1(m, mt):
                return b1sb[m][:, mt : mt + 1]
        else:
            def b1(m, mt):
                return 0.0

        state = [None] * NT  # per-tile handles for the deferred combine

        def emit_x(T):
            x8 = xtp.tile([128, KT, RT], f8, tag="x8")
            nc.sync.dma_start(x8[:], x8_d.ap()[T])
            xth = xtp.tile([128, KT, RT], f16, tag="xth")
            xtl = xtp.tile([128, KT, RT], f16, tag="xtl")
            xtq = xtp.tile([128, KT, RT], f16, tag="xtq")
            nc.sync.dma_start(xth[:], xth_d.ap()[T])
            nc.sync.dma_start(xtl[:], xtl_d.ap()[T])
            nc.gpsimd.dma_start(xtq[:], xtq_d.ap()[T])
            state[T] = dict(xth=xth, xtl=xtl, xtq=xtq, x8=x8)

        def emit_ew_L1(T):
            st = state[T]
            hte = hp.tile([128, KT, RT], f8, tag="hte")
            for mt in range(MT1):
                ms = slice(mt * 128, (mt + 1) * 128)
                ps = psmm.tile([128, RT], f32, tag="psmm")
                for j in range(KT // 2):
                    nc.tensor.matmul(
                        ps[:], w1esb[:, 2 * j : 2 * j + 2, ms],
                        st["x8"][:, 2 * j : 2 * j + 2, :],
                        start=(j == 0), stop=(j == KT // 2 - 1),
                        perf_mode=DR,
                    )
                nc.scalar.activation(
                    hte[:, mt, :], ps[:], getattr(AF, act),
                    bias=b1("e", mt), scale=1.0 / (W8 * X8),
                )
            st["hte"] = hte

        def emit_cls_L1(T):
            st = state[T]
            htc = hp.tile([128, KT, RT], f16, tag="htc")
            for mt in range(MT1):
                ms = slice(mt * 128, (mt + 1) * 128)
                ps = psmm.tile([128, RT], f32, tag="psmm")
                for k in range(KT):
                    nc.tensor.matmul(
                        ps[:], w1csb[:, k, ms], st["xth"][:, k, :],
                        start=(k == 0), stop=(k == KT - 1),
                    )
                nc.scalar.activation(
                    htc[:, mt, :], ps[:], getattr(AF, act), bias=b1("c", mt),
                )
            st["htc"] = htc

        def emit_cls_L2(T):
            """cls head + DVE partial class-sums as activations land."""
            st = state[T]
            htc = st["htc"]
            expP = epp.tile([E, MT2, RT], f16, tag="expP")
            t01 = wp.tile([E, RT], f32, tag="t01")
            t23 = wp.tile([E, RT], f32, tag="t23")
            t04 = wp.tile([E, RT], f32, tag="t04")
            S_sb = wp.tile([E, RT], f32, tag="S_sb")
            for t2 in range(MT2):
                ps = psmm.tile([128, RT], f32, tag="psmm")
                for k in range(KT):
                    nc.tensor.matmul(
                        ps[:E], w2csb[:, k, t2 * E : (t2 + 1) * E], htc[:, k, :],
                        start=(k == 0), stop=(k == KT - 1),
                    )
                nc.scalar.activation(
                    expP[:, t2, :], ps[:E], AF.Exp,
                    bias=b2csb[:, t2 : t2 + 1] if has_bias else 0.0,
                )
                if t2 == 1:
                    nc.vector.tensor_tensor(
                        t01[:], expP[:, 0, :], expP[:, 1, :], OP.add)
                elif t2 == 3:
                    nc.vector.tensor_tensor(
                        t23[:], expP[:, 2, :], expP[:, 3, :], OP.add)
                    nc.vector.tensor_tensor(t01[:], t01[:], t23[:], OP.add)
                elif t2 == 4:
                    nc.vector.tensor_tensor(
                        t04[:], t01[:], expP[:, 4, :], OP.add)
                elif t2 == 5:
                    nc.vector.tensor_tensor(
                        S_sb[:], t04[:], expP[:, 5, :], OP.add)
            st["expP"] = expP
            st["S_sb"] = S_sb

        def emit_we(T):
            """which_expert L1 (3-term fp16 split) + L2 (native fp32)."""
            st = state[T]
            xth, xtl, xtq = st["xth"], st["xtl"], st["xtq"]
            htw = hp.tile([128, KT, RT], f32, tag="htw")
            for mt in range(MT1):
                ms = slice(mt * 128, (mt + 1) * 128)
                ps = psmm.tile([128, RT], f32, tag="psmm")
                terms = [(xth, w1wh), (xtl, w1wh), (xtq, w1wl)]
                for k in range(KT):
                    for ti, (xt, wt) in enumerate(terms):
                        nc.tensor.matmul(
                            ps[:], wt[:, k, ms], xt[:, k, :],
                            start=(ti == 0 and k == 0),
                            stop=(ti == 2 and k == KT - 1),
                        )
                nc.scalar.activation(
                    htw[:, mt, :], ps[:], getattr(AF, act), bias=b1("w", mt),
                )
            weT = wep.tile([E, RT], f32, tag="weT")
            ps = psmm.tile([128, RT], f32, tag="psmm")
            for k in range(KT):
                nc.tensor.matmul(
                    ps[:E], w2wsb[:, k, :], htw[:, k, :],
                    start=(k == 0), stop=(k == KT - 1),
                )
            if has_bias:
                nc.vector.tensor_scalar(weT[:], ps[:E], b2wsb[:], None, OP.add)
            else:
                nc.vector.tensor_copy(weT[:], ps[:E])
            st["htw"] = htw
            st["weT"] = weT

        def emit_sort(T):
            """Row-major transposes + per-row top-n threshold sort."""
            st = state[T]
            weT = st["weT"]
            maskRows = []
            for j in range(RT // SUB):
                s = (RT // SUB) * T + j
                Rr, desc = sub_r[s], sub_dir[s]
                cs = slice(j * SUB, (j + 1) * SUB)
                trp = pstr.tile([128, 128], f32, tag="trp")
                nc.tensor.transpose(trp[:, :E], weT[:, cs], ident[:E, :E])
                weRow = sp.tile([128, E], f32, tag="weRow")
                nc.vector.tensor_copy(weRow[:], trp[:, :E])
                scratch = sp.tile([128, E], f32, tag="scratch")
                if desc:
                    nc.vector.tensor_copy(scratch[:], trp[:, :E])
                else:
                    nc.vector.tensor_scalar_mul(scratch[:], trp[:, :E], -1.0)
                srt = sp.tile([128, 128], f32, tag="srt")
                for r in range(Rr):
                    nc.vector.max(out=srt[:, 8 * r : 8 * r + 8], in_=scratch[:])
                    if r < Rr - 1:
                        nc.vector.match_replace(
                            out=scratch[:],
                            in_to_replace=srt[:, 8 * r : 8 * r + 8],
                            in_values=scratch[:],
                            imm_value=NEG_FILL,
                        )
                w8 = 8 * Rr
                ohtmp = sp.tile([128, 128], f32, tag="ohtmp")
                thr = sp.tile([128, 1], f32, tag="thr")
                nc.vector.scalar_tensor_tensor(
                    out=ohtmp[:, :w8],
                    in0=iotam[:, :w8],
                    scalar=kselsb[:, s : s + 1],
                    in1=srt[:, :w8],
                    op0=OP.is_equal,
                    op1=OP.mult,
                    accum_out=thr[:],
                )
                if not desc:
                    nc.vector.tensor_scalar_mul(thr[:], thr[:], -1.0)
                maskRow = mrp.tile([128, E], f16, tag="maskRow")
                nc.vector.tensor_scalar(maskRow[:], weRow[:], thr[:], None, OP.is_ge)
                maskRows.append(maskRow)
            st["maskRows"] = maskRows

        def emit_ew_L2(T):
            st = state[T]
            expew = wep.tile([E, RT], f16, tag="expew")
            ps = psmm.tile([128, RT], f32, tag="psmm")
            for j in range(KT // 2):
                nc.tensor.matmul(
                    ps[:], w2esb[:, 2 * j : 2 * j + 2, :],
                    st["hte"][:, 2 * j : 2 * j + 2, :],
                    start=(j == 0), stop=(j == KT // 2 - 1),
                    perf_mode=DR,
                )
            nc.scalar.activation(
                expew[:], ps[:E], AF.Exp,
                bias=b2esb[:] if has_bias else 0.0, scale=1.0 / W8,
            )
            st["expew"] = expew

        def emit_maskT(T):
            st = state[T]
            maskT = psmask.tile([E, RT], f16, tag="maskT")
            for j in range(RT // SUB):
                cs = slice(j * SUB, (j + 1) * SUB)
                nc.tensor.transpose(maskT[:, cs], st["maskRows"][j][:], ident16[:])
            st["maskT"] = maskT

        def emit_D(T):
            """Combine: mask weights, class softmax, weighted sum; the
            normalization by the expert-weight sum happens on HOST."""
            st = state[T]
            rs = slice(T * RT, (T + 1) * RT)
            expP, expew, maskT = st["expP"], st["expew"], st["maskT"]
            wT = wp.tile([E, RT], f16, tag="wT")
            nc.vector.tensor_tensor(wT[:], expew[:], maskT[:], OP.mult)
            den_ps = pss.tile([E, RT], f32, tag="S")
            nc.tensor.matmul(den_ps[:1, :], ones124[:], wT[:], start=True, stop=True)
            den_sb = wp.tile([1, RT], f32, tag="den_sb")
            nc.vector.tensor_copy(den_sb[:], den_ps[:1, :])
            nc.sync.dma_start(den_d.ap()[:, rs], den_sb[:])
            Sr = wp.tile([E, RT], f32, tag="Sr")
            nc.vector.reciprocal_approx_fast(Sr[:], st["S_sb"][:])
            u = wp.tile([E, RT], f16, tag="u")
            nc.vector.tensor_tensor(u[:], wT[:], Sr[:], OP.mult)
            out_ps = psout.tile([C, RT], f32, tag="out")
            for t2 in range(MT2):
                wexp = wp.tile([E, RT], f16, tag=f"wexp{t2 % 5}")
                eng = nc.gpsimd if t2 == 5 else nc.vector
                eng.tensor_tensor(wexp[:], expP[:, t2, :], u[:], OP.mult)
                nc.tensor.matmul(
                    out_ps[:], hmat[:, t2, :], wexp[:],
                    start=(t2 == 0), stop=(t2 == MT2 - 1),
                )
            out_sb = wp.tile([C, RT], f32, tag="out_sb")
            nc.vector.tensor_copy(out_sb[:], out_ps[:])
            nc.sync.dma_start(out_d.ap()[:, rs], out_sb[:])
            state[T] = None

        # Tile 0 in data-arrival order, then the steady-state pipeline.
        # C-phase order ... -> cls L2 -> ew L2 -> maskT keeps the long
        # serial combine chain (S -> 1/S -> u) off the final-tile tail.
        emit_x(0)
        w2csb = load_w(nc.gpsimd, w2c_d, EC, f16, "w2c")
        w1wl = load_w(nc.gpsimd, w1wl_d, D, f16, "w1wl")
        ident = load_c(nc.gpsimd, ident_d, [128, 128], f32, "ident")
        iotam = load_c(nc.gpsimd, iota_d, [128, 128], f32, "iota")
        kselsb = load_c(nc.gpsimd, ksel_d, [SUB, NS], f32, "ksel")
        w2wsb = load_w(nc.gpsimd, w2w_d, E, f32, "w2w")
        w2esb = load_w(nc.gpsimd, w2e_d, 128, f8, "w2e")
        ident16 = load_c(nc.gpsimd, ident16_d, [128, 128], f16, "ident16")
        hmat = load_c(nc.gpsimd, hmat_d, [E, MT2, C], f16, "hmat")
        emit_ew_L1(0)
        emit_cls_L1(0)
        emit_cls_L2(0)
        emit_we(0)
        emit_sort(0)
        emit_ew_L2(0)
        emit_maskT(0)
        for T in range(1, NT):
            emit_x(T)
            emit_we(T)
            emit_sort(T)
            emit_D(T - 1)
            emit_ew_L1(T)
            emit_cls_L1(T)
            emit_cls_L2(T)
            emit_ew_L2(T)
            emit_maskT(T)
        emit_D(NT - 1)

    nc.compile()
    return nc


def _get_nc(sub_dir, sub_r, act="Gelu", has_bias=False):
    key = (tuple(sub_dir), tuple(sub_r), act, has_bias)
    if key not in _BUILD_CACHE:
        _BUILD_CACHE[key] = _build_nc(sub_dir, sub_r, act, has_bias)
    return _BUILD_CACHE[key]


def _host_prep(x, n_experts):
    n = np.minimum(np.asarray(n_experts).astype(np.int64), E).astype(np.int32)
    order = np.argsort(n, kind="stable")
    ns_sorted = n[order]

    sub_dir, sub_r = SUB_DIR, SUB_R
    ok = True
    for s in range(NS):
        lo = int(ns_sorted[(B // NS) * s])
        hi = int(ns_sorted[(B // NS) * (s + 1) - 1])
        if sub_dir[s]:
            ok &= hi <= 8 * sub_r[s]
        else:
            ok &= lo >= E + 1 - 8 * sub_r[s]
    if not ok:
        sub_dir, sub_r = FALLBACK_DIR, FALLBACK_R

    rows_by_core = [order[c::NCORES] for c in range(NCORES)]
    xts, ksels = [], []
    for c in range(NCORES):
        rows = rows_by_core[c]
        xts.append(np.ascontiguousarray(x[rows].T.astype(np.float32)))
        nv = n[rows].astype(np.float32)
        ks = np.empty(BC, np.float32)
        for s in range(NS):
            seg = slice(SUB * s, SUB * (s + 1))
            ks[seg] = (nv[seg] - 1.0) if sub_dir[s] else (E - nv[seg])
        ksels.append(np.ascontiguousarray(ks.reshape(NS, SUB).T))
    return rows_by_core, xts, ksels, sub_dir, sub_r


def _cls_perm():
    """perm[t2 * E + e] = e * C + t2 : slot (e, t2) holds (expert e, class t2)."""
    t2, e = np.meshgrid(np.arange(MT2), np.arange(E), indexing="ij")
    return (e * C + t2).reshape(-1)


def _host_consts():
    hmat = np.zeros((E, MT2, C), np.float32)
    for t in range(MT2):
        hmat[:, t, t] = 1.0
    ident = np.eye(128, dtype=np.float32)
    iota = np.broadcast_to(np.arange(128, dtype=np.float32), (128, 128)).copy()
    return hmat, ident, iota


def _has_bias(inputs):
    return any(
        np.any(np.asarray(inputs[k]))
        for k in ("cls_b1", "cls_b2", "we_b1", "we_b2", "ew_b1", "ew_b2")
    )


def _host_inputs(inputs, has_bias):
    """All DRAM input arrays except the per-core x/ksel."""
    import ml_dtypes

    f16 = np.float16
    f8 = ml_dtypes.float8_e4m3
    f32 = np.float32
    hmat, ident, iota = _host_consts()
    perm = _cls_perm()

    def pm(w):
        # [D, cols] -> partition-major [128, KT, cols]: each partition's
        # KT*cols elements are one contiguous DMA run.
        return np.ascontiguousarray(
            w.reshape(KT, 128, w.shape[1]).transpose(1, 0, 2))

    w1w = np.asarray(inputs["we_w1"], f32)
    w1wh = w1w.astype(f16)
    w1wl = ((w1w - w1wh.astype(f32)) * LO).astype(f16)
    w2c = np.asarray(inputs["cls_w2"], f32)[:, perm]
    feeds = {
        "w1c": pm(np.asarray(inputs["cls_w1"], f32).astype(f16)),
        "w1wh": pm(w1wh),
        "w1wl": pm(w1wl),
        "w1e": pm((np.asarray(inputs["ew_w1"], f32) * W8).astype(f8)),
        "w2c": pm(w2c.astype(f16)),
        "w2w": pm(np.asarray(inputs["we_w2"], f32)),
        "w2e": pm(np.concatenate(
            [(np.asarray(inputs["ew_w2"], f32) * W8).astype(f8),
             np.zeros((D, 128 - E), f8)], axis=1)),
        "hmat": hmat.astype(f16),
        "ident": ident,
        "ident16": ident.astype(f16),
        "iota": iota,
    }
    if has_bias:
        b2c = np.asarray(inputs["cls_b2"], f32)[perm]
        feeds.update({
            "b1c": np.ascontiguousarray(
                np.asarray(inputs["cls_b1"], f32).reshape(MT1, 128).T),
            "b1w": np.ascontiguousarray(
                np.asarray(inputs["we_b1"], f32).reshape(MT1, 128).T),
            "b1e": np.ascontiguousarray(
                np.asarray(inputs["ew_b1"], f32).reshape(MT1, 128).T),
            "b2c": np.ascontiguousarray(b2c.reshape(MT2, E).T),
            "b2w": np.asarray(inputs["we_b2"], f32).reshape(E, 1),
            "b2e": np.asarray(inputs["ew_b2"], f32).reshape(E, 1),
        })
    return feeds


def _tile_major(a):
    # [D, BC] -> [NT, 128, KT, RT]: per (tile, partition) one contiguous run.
    return np.ascontiguousarray(
        a.reshape(KT, 128, NT, RT).transpose(2, 1, 0, 3))


def _per_core_inputs(xts, ksels, c):
    import ml_dtypes

    f16 = np.float16
    xt = xts[c]                      # [D, BC] fp32
    xth = xt.astype(f16)
    xtl = (xt - xth.astype(np.float32)).astype(f16)
    xtq = (xth.astype(np.float32) / LO).astype(f16)
    x8 = (xt * X8).astype(ml_dtypes.float8_e4m3)
    return {"xth": _tile_major(xth), "xtl": _tile_major(xtl),
            "xtq": _tile_major(xtq), "x8": _tile_major(x8),
            "ksel": ksels[c]}


def kernel(**inputs):
    x = np.asarray(inputs["x"], np.float32)
    rows_by_core, xts, ksels, sub_dir, sub_r = _host_prep(x, inputs["n_experts"])
    has_bias = _has_bias(inputs)
    shared = _host_inputs(inputs, has_bias)
    in_maps = [
        {**shared, **_per_core_inputs(xts, ksels, c)} for c in range(NCORES)
    ]

    nc = _get_nc(sub_dir, sub_r, has_bias=has_bias)

    from concourse.bass_utils import run_bass_kernel_spmd

    res = run_bass_kernel_spmd(nc, in_maps, core_ids=list(range(NCORES)))

    full = np.empty((B, C), np.float32)
    for c in range(NCORES):
        r = res.results[c]
        full[rows_by_core[c]] = (r["out"] / r["den"]).T
    return full


if __name__ == "__main__":
    print("smoke build only")
    _get_nc(SUB_DIR, SUB_R)
    print("built ok")
